# revision 22
# baseline (speedup 1.0000x reference)
"""Trainium2 Bass kernel for ContrastiveAffinityLossWithMemoryV2.

Math: with MARGIN=4 and d = ||a-b|| <= 2 for unit vectors, relu(M-d) = M-d,
so each pairwise loss term simplifies:
    t*d^2 + (1-t)*(M-d)^2 = d^2 + (1-t)*(16 - 8*d)
Sum(d^2) and Sum(1-t) are *linear* and evaluated exactly on host from vector
sums; the only part needing the full B x B pair plane / B x C memory plane is
    P3 = Sum 8*d * (1-t)
which the device computes, sharded over 8 NeuronCores:
  - PE: one fp8 DoubleRow matmul per 128x512 unit (two 96-row k-tiles cover
    D=192 in a single pass at 0.5 cycles/row); rhs pre-scaled by -2 (exact in
    fp8), so psum = -2*S
  - ScalarE: d8 = sqrt(64*psum + 128 + delta) = 8*d. Embeddings are truncated
    to fp8 *toward zero*, so every row norm stays <= 1 and the sqrt argument
    is structurally non-negative.
  - VectorE: scalar_tensor_tensor fused multiply+reduce against host-shipped
    fp8 masks (stochastically rounded so quantization is unbiased), one STT
    per pair of psum groups to amortize instruction overhead.
The pair plane is computed only for j > i: row-blocks are dealt to cores so
every core owns exactly 18 of the 144 upper-triangle (row-block x 512-chunk)
units; per-unit operands are duplicated into flat arrays so all cores run the
same program on different data. DMA issues are spread across the Sync,
Activation and GpSimd queues in consumption order so transfers overlap
compute. Host combines partials with the closed-form terms.
"""

import numpy as np
import ml_dtypes

N_CLASSES = 8192
B = 4096
D = 192  # 256 * 0.75
KP = 96  # k-tile partitions (2 k-tiles of 96 cover D=192)
NCORES = 8
ROWS = B // NCORES          # 512 rows per core
NRB = B // 128              # 32 global row-blocks
MARGIN = 4.0
MEMORY_WEIGHT = 0.5
WARMUP_STEPS = 1000
MOM_WARMUP = 5000
BASE_MOM = 0.9
BG_SIM = 0.2
BG_OTHER_SIM = 0.01
EPS = 1e-12
DELTA2 = 0.01
NGU = 18                    # G-plane units per core (144 / 8)

bf16 = ml_dtypes.bfloat16
f8 = ml_dtypes.float8_e4m3

# row-block deal: cores 0-3 get chunk-counts {8,7,2,1}, cores 4-7 {6,5,4,3}
CORE_RBS = [[k, 4 + k, 24 + k, 28 + k] for k in range(4)] + \
           [[8 + k, 12 + k, 16 + k, 20 + k] for k in range(4)]


def _g_chunks(rb):
    """512-col chunks containing any j > i for row-block rb."""
    return [cc for cc in range(8) if 512 * cc + 511 >= 128 * rb + 1]


_CACHE = {}


def trunc_f8(v):
    """fp32 -> fp8 e4m3 rounded toward zero: row L2 norms can only shrink."""
    x = np.ascontiguousarray(v, dtype=np.float32)
    y = x.astype(f8)
    yb = y.view(np.uint8).copy()
    over = np.abs(y.astype(np.float32)) > np.abs(x)
    yb[over & ((yb & 0x7F) > 0)] -= 1
    return yb.view(f8)


def stoch_fp8(v, seed):
    """Stochastic rounding to float8_e4m3 (values >= 0)."""
    x = np.ascontiguousarray(v, dtype=np.float32)
    y = x.astype(f8)
    yb = y.view(np.uint8).copy()
    over = np.abs(y.astype(np.float32)) > x
    yb[over & ((yb & 0x7F) > 0)] -= 1
    fl = yb.view(f8)
    ce = (yb + (fl.astype(np.float32) < x).astype(np.uint8)).view(f8)
    flf = fl.astype(np.float32)
    gap = ce.astype(np.float32) - flf
    p = np.where(gap > 0, (x - flf) / np.where(gap > 0, gap, 1.0), 0.0)
    rng = np.random.default_rng(seed)
    up = rng.random(x.shape, dtype=np.float32) < p
    return np.where(up, ce, fl).astype(f8)


def _bank_chains(zn, y_true, momentum):
    """Replicate the reference's sequential per-sample EMA scatter (fp32)."""
    valid = (y_true >= 0) & (y_true < N_CLASSES)
    lc = np.clip(y_true, 0, N_CLASSES - 1)
    m = np.float32(momentum)
    one_m = np.float32(1.0 - momentum)
    bank = {}
    for i in np.nonzero(valid)[0]:
        c = int(lc[i])
        if c not in bank:
            bank[c] = zn[i].copy()
        else:
            ema = m * bank[c] + one_m * zn[i]
            n = np.float32(np.sqrt(np.float32((ema ** 2).sum())))
            bank[c] = ema / max(n, np.float32(EPS))
    return bank


def _build_nc(CS):
    """CS = number of 512-wide S-plane chunks (CP = 512*CS classes)."""
    from concourse import bacc, tile, mybir

    dt = mybir.dt
    CP = 512 * CS
    nc = bacc.Bacc("TRN2", target_bir_lowering=False, debug=False)

    zl_d = nc.dram_tensor("zl", (KP, 2 * ROWS), dt.float8e4, kind="ExternalInput")
    rs_d = nc.dram_tensor("rs", (KP, 2 * CP), dt.float8e4, kind="ExternalInput")
    lg_d = nc.dram_tensor("lg", (KP, 2 * NGU * 128), dt.float8e4, kind="ExternalInput")
    rg_d = nc.dram_tensor("rg", (KP, 2 * NGU * 512), dt.float8e4, kind="ExternalInput")
    r1_d = nc.dram_tensor("r1", (128, 4 * CP), dt.float8e4, kind="ExternalInput")
    t2_d = nc.dram_tensor("t2", (128, NGU * 512), dt.float8e4, kind="ExternalInput")
    out_d = nc.dram_tensor("acc_out", (128, 16), dt.float32, kind="ExternalOutput")

    # unit list: (lhs tensor key, lhs col, rhs col, mask col)
    units = []
    for ib in range(4):
        for cc in range(CS):
            units.append(("s", ib * 128, cc * 512, (ib * CS + cc) * 512))
    for u in range(NGU):
        units.append(("g", u * 128, u * 512, u * 512))
    n_units = len(units)
    GSZ = 3                                # units per psum group (3 banks)
    n_groups = (n_units + GSZ - 1) // GSZ

    DR = mybir.MatmulPerfMode.DoubleRow

    with tile.TileContext(nc) as tc:
        with (
            tc.tile_pool(name="const", bufs=1) as constp,
            tc.tile_pool(name="d8p", bufs=3) as d8p,
            tc.tile_pool(name="ep", bufs=3) as ep,
            tc.tile_pool(name="accp", bufs=1) as accp,
            tc.tile_pool(name="psp", bufs=2, space="PSUM") as psp,
            tc.tile_pool(name="pswp", bufs=1, space="PSUM") as pswp,
        ):
            # resident input tiles ([KP, 2, N] = two k-tiles along free axis)
            zl = constp.tile([KP, 2, ROWS], dt.float8e4, tag="zl")
            rs = constp.tile([KP, 2, CP], dt.float8e4, tag="rs")
            lg = constp.tile([KP, 2, NGU * 128], dt.float8e4, tag="lg")
            rg = constp.tile([KP, 2, NGU * 512], dt.float8e4, tag="rg")
            r1 = constp.tile([128, 4 * CP], dt.float8e4, tag="r1")
            t2 = constp.tile([128, NGU * 512], dt.float8e4, tag="t2")

            bias_t = constp.tile([128, 1], dt.float32)
            acc_all = accp.tile([128, 16], dt.float32)
            nc.gpsimd.memset(bias_t[:], 128.0 + float(DELTA2))
            nc.gpsimd.memset(acc_all[:], 0.0)

            # --- DMA issue ---
            # Transfers share the 16 DMA engines, so concurrent bulk streams
            # starve the critical path; keep one consumption-ordered chain on
            # Sync and only the group-0 critical operands on Scalar.
            # group-0/1 critical operands first, in small chunks
            c0w = min(2048, CP)
            nc.sync.dma_start(rs[:, 0, 0:c0w], rs_d[:, 0:c0w])
            nc.sync.dma_start(rs[:, 1, 0:c0w], rs_d[:, CP:CP + c0w])
            nc.sync.dma_start(zl[:], zl_d[:].rearrange("p (t n) -> p t n", t=2))
            nc.sync.dma_start(r1[:, 0:2048], r1_d[:, 0:2048])
            if c0w < CP:
                nc.sync.dma_start(rs[:, 0, c0w:CP], rs_d[:, c0w:CP])
                nc.sync.dma_start(rs[:, 1, c0w:CP], rs_d[:, CP + c0w:2 * CP])
            nc.sync.dma_start(r1[:, 2048:4096], r1_d[:, 2048:4096])
            # G-plane matmul operands next: the PE reaches them long before
            # the later mask spans are consumed (keeps the HAM clock warm)
            GW = NGU * 512
            nc.sync.dma_start(lg[:], lg_d[:].rearrange("p (t n) -> p t n", t=2))
            nc.sync.dma_start(rg[:, 0, 0:4096], rg_d[:, 0:4096])
            nc.sync.dma_start(rg[:, 1, 0:4096], rg_d[:, GW:GW + 4096])
            nc.sync.dma_start(r1[:, 4096:8192], r1_d[:, 4096:8192])
            nc.sync.dma_start(rg[:, 0, 4096:GW], rg_d[:, 4096:GW])
            nc.sync.dma_start(rg[:, 1, 4096:GW], rg_d[:, GW + 4096:2 * GW])
            nc.sync.dma_start(r1[:, 8192:4 * CP], r1_d[:, 8192:4 * CP])
            nc.sync.dma_start(t2[:, 0:4096], t2_d[:, 0:4096])
            nc.sync.dma_start(t2[:, 4096:8192], t2_d[:, 4096:8192])
            nc.sync.dma_start(t2[:, 8192:GW], t2_d[:, 8192:GW])

            # PE warm-up: dummy matmuls on scratch data while real operands
            # stream in, so the HAM clock gate is already ramped when group 0
            # lands; small fillers between groups keep the duty cycle high.
            warm = constp.tile([KP, 2, 512], dt.float8e4, tag="warm")
            nc.gpsimd.memset(warm[:], 0.0)
            psw = pswp.tile([128, 512], dt.float32, tag="psw")
            for wi in range(10):
                nc.tensor.matmul(
                    psw[:], warm[:, :, 0:128], warm[:],
                    start=True, stop=True, perf_mode=DR,
                )

            ops = {"s": (zl, rs, r1), "g": (lg, rg, t2)}
            d8t = None
            d8_off = 0
            span_key = None
            span_m0 = 0
            si = 0
            n_s_spans = 0
            for gi in range(n_groups):
                gunits = units[gi * GSZ:(gi + 1) * GSZ]
                gw = 512 * len(gunits)
                key = gunits[0][0]
                ps = psp.tile([128, 512 * GSZ], dt.float32, tag="ps")
                for q, (_, lc0, rc0, mc0) in enumerate(gunits):
                    lt, rt, _ = ops[key]
                    nc.tensor.matmul(
                        ps[:, q * 512:(q + 1) * 512],
                        lt[:, :, lc0:lc0 + 128],
                        rt[:, :, rc0:rc0 + 512],
                        start=True, stop=True, perf_mode=DR,
                    )
                for _f in range(2):
                    nc.tensor.matmul(
                        psw[:, 0:128], warm[:, :, 0:128], warm[:, :, 0:128],
                        start=True, stop=True, perf_mode=DR,
                    )
                if d8t is None:
                    d8t = d8p.tile([128, 3072], dt.bfloat16, tag="d8")
                    d8_off = 0
                    span_key = key
                    span_m0 = gunits[0][3]
                nc.scalar.activation(
                    d8t[:, d8_off:d8_off + gw], ps[:, 0:gw],
                    mybir.ActivationFunctionType.Sqrt,
                    bias=bias_t[:], scale=64.0,
                )
                d8_off += gw
                nxt = units[(gi + 1) * GSZ:(gi + 1) * GSZ + 1]
                # close the first S span early so VectorE starts sooner
                close = (d8_off >= 3072) or (not nxt) or (nxt[0][0] != span_key) \
                    or (si < 1 and d8_off >= 1536)
                if close:
                    mask = ops[span_key][2]
                    et = ep.tile([128, 3072], dt.bfloat16, tag="et")
                    nc.vector.scalar_tensor_tensor(
                        out=et[:, 0:d8_off],
                        in0=d8t[:, 0:d8_off],
                        scalar=1.0,
                        in1=mask[:, span_m0:span_m0 + d8_off],
                        op0=mybir.AluOpType.mult,
                        op1=mybir.AluOpType.mult,
                        accum_out=acc_all[:, si:si + 1],
                    )
                    if span_key == "s":
                        n_s_spans += 1
                    si += 1
                    d8t = None
            n_spans = si
            assert n_spans <= 16

            nc.sync.dma_start(out_d[:], acc_all[:])

    nc.compile()
    return nc, n_spans, n_s_spans


def _get_nc(CS):
    key = ("nc", CS)
    if key not in _CACHE:
        _CACHE[key] = _build_nc(CS)
    return _CACHE[key]


def kernel(y_true, y_pred, lookup, global_step, current_epoch, _want_trace=False):
    from concourse.bass_utils import run_bass_kernel_spmd

    y_true = np.asarray(y_true).astype(np.int64)
    y_pred = np.asarray(y_pred, dtype=np.float32)
    lookup = np.asarray(lookup, dtype=np.float32)
    gs = int(np.asarray(global_step))

    if gs < MOM_WARMUP:
        momentum = 0.5 + (BASE_MOM - 0.5) * (gs / MOM_WARMUP)
    else:
        momentum = BASE_MOM
    progress = min(1.0, (gs - WARMUP_STEPS) / 5000.0)
    aw = MEMORY_WEIGHT * progress

    # ---- host: normalize, bank scatter-EMA, compaction ----
    z = y_pred[:, :D]
    nrm = np.sqrt((z.astype(np.float64) ** 2).sum(axis=1))
    zn = (z / np.maximum(nrm, EPS)[:, None]).astype(np.float32)

    valid = (y_true >= 0) & (y_true < N_CLASSES)
    bg = ~valid
    nv = int(valid.sum())
    lc = np.clip(y_true, 0, N_CLASSES - 1)

    bank = _bank_chains(zn, y_true, momentum)
    init_list = np.array(sorted(bank.keys()), dtype=np.int64)
    C = len(init_list)
    CS = max(1, (C + 511) // 512)
    CP = 512 * CS

    zn8 = trunc_f8(zn)
    bank_rows = (
        np.stack([bank[c] for c in init_list])
        if C else np.zeros((0, D), np.float32)
    )
    bank8 = trunc_f8(bank_rows)

    znd = zn8.astype(np.float64)
    bankd = bank8.astype(np.float64)

    # ---- host: exact linear terms (fp64) ----
    R = lookup[lc]                    # (B, 8192)
    R_init = R[:, init_list]          # (B, C)
    A_S = 2.0 * nv * C - 2.0 * float(znd[valid].sum(0) @ bankd.sum(0))
    B_S = nv * C - float(R_init[valid].sum(dtype=np.float64))

    T_up = R[:, lc]                   # (B, B): lookup[lc_i, lc_j]
    both_bg = bg[:, None] & bg[None, :]
    one_bg = bg[:, None] ^ bg[None, :]
    T_up = np.where(both_bg, np.float32(BG_SIM),
                    np.where(one_bg, np.float32(BG_OTHER_SIM), T_up))
    # upper-triangle (i<j) oriented pair targets; zero elsewhere
    T_up = np.triu(T_up, 1)

    Np = B * (B - 1) // 2
    szn = znd.sum(0)
    sumG_offdiag = float(szn @ szn) - float((znd ** 2).sum())
    A_G = 2.0 * Np - sumG_offdiag
    B_G = Np - float(T_up.sum(dtype=np.float64))

    # ---- device operand construction (fp8, two 96-row k-tiles) ----
    znT8 = np.ascontiguousarray(zn8.T)                       # (192, B) f8
    znT8m2 = (zn8.astype(np.float32).T * np.float32(-2.0)).astype(f8)
    bankT8m2 = np.zeros((D, CP), dtype=f8)
    if C:
        bankT8m2[:, 0:C] = (
            bank8.astype(np.float32).T * np.float32(-2.0)
        ).astype(f8)

    def ktiles(a):
        """(192, N) -> (96, 2*N): two k-tiles along the free axis."""
        return np.ascontiguousarray(
            np.concatenate([a[0:KP], a[KP:2 * KP]], axis=1)
        )

    in_maps = []
    for core in range(NCORES):
        rbs = CORE_RBS[core]
        rows = np.concatenate([np.arange(rb * 128, rb * 128 + 128) for rb in rbs])

        zl = ktiles(znT8[:, rows])                           # (96, 1024)
        rs = ktiles(bankT8m2)                                # (96, 2*CP)
        r1 = np.zeros((128, 4 * CP), dtype=f8)
        for ib, rb in enumerate(rbs):
            rr = slice(rb * 128, rb * 128 + 128)
            m = (1.0 - R_init[rr]) * valid[rr, None]         # (128, C)
            r1[:, ib * CP:ib * CP + C] = stoch_fp8(m, seed=1000 + rb)

        gunits = [(ib, rb, cc) for ib, rb in enumerate(rbs)
                  for cc in _g_chunks(rb)]
        assert len(gunits) == NGU, (core, len(gunits))

        lgf = np.empty((D, NGU * 128), dtype=f8)
        rgf = np.empty((D, NGU * 512), dtype=f8)
        t2 = np.zeros((128, NGU * 512), dtype=f8)
        for u, (ib, rb, cc) in enumerate(gunits):
            lgf[:, u * 128:(u + 1) * 128] = znT8[:, rb * 128:rb * 128 + 128]
            rgf[:, u * 512:(u + 1) * 512] = znT8m2[:, cc * 512:(cc + 1) * 512]
            blk = 1.0 - T_up[rb * 128:rb * 128 + 128, cc * 512:(cc + 1) * 512]
            jj = np.arange(cc * 512, cc * 512 + 512)[None, :]
            ii = np.arange(rb * 128, rb * 128 + 128)[:, None]
            blk = np.where(jj > ii, blk, 0.0)
            t2[:, u * 512:(u + 1) * 512] = stoch_fp8(blk, seed=2000 + rb * 8 + cc)

        in_maps.append({
            "zl": zl,
            "rs": rs,
            "lg": ktiles(lgf),
            "rg": ktiles(rgf),
            "r1": r1,
            "t2": t2,
        })

    nc, n_spans, n_s_spans = _get_nc(CS)
    if _want_trace:
        import tempfile
        try:
            from trn_agent_boot.trn_boot import _ntff_profile_via_ctypes
            hook = _ntff_profile_via_ctypes("/opt/axon/libaxon_pjrt.so")
            outdir = tempfile.mkdtemp(prefix="ntff_")
            with hook(outdir, [0]):
                res = run_bass_kernel_spmd(nc, in_maps, list(range(NCORES)))
            _CACHE["last_profile_dir"] = outdir
        except Exception as e:
            _CACHE["trace_error"] = repr(e)
            res = run_bass_kernel_spmd(nc, in_maps, list(range(NCORES)))
        _CACHE["last_results"] = res
    else:
        res = run_bass_kernel_spmd(nc, in_maps, list(range(NCORES)))

    P3S = 0.0
    P3G = 0.0
    for r in res.results:
        acc = np.asarray(r["acc_out"], dtype=np.float64)
        P3S += float(acc[:, 0:n_s_spans].sum())
        P3G += float(acc[:, n_s_spans:n_spans].sum())

    mem_sum = A_S + 16.0 * B_S - P3S
    denom = max(nv * C, 1)
    mem_loss = mem_sum / denom
    batch_sum = A_G + 16.0 * B_G - P3G
    batch_loss = batch_sum / Np

    loss = (1.0 - aw) * batch_loss + aw * mem_loss
    return np.float32(loss)


# revision 23
# speedup vs baseline: 1.0848x; 1.0848x over previous
"""Trainium2 Bass kernel for ContrastiveAffinityLossWithMemoryV2.

Math: with MARGIN=4 and d = ||a-b|| <= 2 for unit vectors, relu(M-d) = M-d,
so each pairwise loss term simplifies:
    t*d^2 + (1-t)*(M-d)^2 = d^2 + (1-t)*(16 - 8*d)
Sum(d^2) and Sum(1-t) are *linear* and evaluated exactly on host from vector
sums; the only part needing the full B x B pair plane / B x C memory plane is
    P3 = Sum 8*d * (1-t)
which the device computes, sharded over 8 NeuronCores:
  - PE: one fp8 DoubleRow matmul per 128x512 unit (two 96-row k-tiles cover
    D=192 in a single pass at 0.5 cycles/row); rhs pre-scaled by -2 (exact in
    fp8), so psum = -2*S
  - ScalarE: d8 = sqrt(64*psum + 128 + delta) = 8*d. Embeddings are truncated
    to fp8 *toward zero*, so every row norm stays <= 1 and the sqrt argument
    is structurally non-negative.
  - VectorE: scalar_tensor_tensor fused multiply+reduce against host-shipped
    fp8 masks (stochastically rounded so quantization is unbiased), one STT
    per pair of psum groups to amortize instruction overhead.
The pair plane is computed only for j > i: row-blocks are dealt to cores so
every core owns exactly 18 of the 144 upper-triangle (row-block x 512-chunk)
units; per-unit operands are duplicated into flat arrays so all cores run the
same program on different data. DMA issues are spread across the Sync,
Activation and GpSimd queues in consumption order so transfers overlap
compute. Host combines partials with the closed-form terms.
"""

import numpy as np
import ml_dtypes

N_CLASSES = 8192
B = 4096
D = 192  # 256 * 0.75
KP = 96  # k-tile partitions (2 k-tiles of 96 cover D=192)
NCORES = 8
ROWS = B // NCORES          # 512 rows per core
NRB = B // 128              # 32 global row-blocks
MARGIN = 4.0
MEMORY_WEIGHT = 0.5
WARMUP_STEPS = 1000
MOM_WARMUP = 5000
BASE_MOM = 0.9
BG_SIM = 0.2
BG_OTHER_SIM = 0.01
EPS = 1e-12
DELTA2 = 0.01
NGU = 18                    # G-plane units per core (144 / 8)

bf16 = ml_dtypes.bfloat16
f8 = ml_dtypes.float8_e4m3

# row-block deal: cores 0-3 get chunk-counts {8,7,2,1}, cores 4-7 {6,5,4,3}
CORE_RBS = [[k, 4 + k, 24 + k, 28 + k] for k in range(4)] + \
           [[8 + k, 12 + k, 16 + k, 20 + k] for k in range(4)]


def _g_chunks(rb):
    """512-col chunks containing any j > i for row-block rb."""
    return [cc for cc in range(8) if 512 * cc + 511 >= 128 * rb + 1]


_CACHE = {}


def trunc_f8(v):
    """fp32 -> fp8 e4m3 rounded toward zero: row L2 norms can only shrink."""
    x = np.ascontiguousarray(v, dtype=np.float32)
    y = x.astype(f8)
    yb = y.view(np.uint8).copy()
    over = np.abs(y.astype(np.float32)) > np.abs(x)
    yb[over & ((yb & 0x7F) > 0)] -= 1
    return yb.view(f8)


def stoch_fp8(v, seed):
    """Stochastic rounding to float8_e4m3 (values >= 0)."""
    x = np.ascontiguousarray(v, dtype=np.float32)
    y = x.astype(f8)
    yb = y.view(np.uint8).copy()
    over = np.abs(y.astype(np.float32)) > x
    yb[over & ((yb & 0x7F) > 0)] -= 1
    fl = yb.view(f8)
    ce = (yb + (fl.astype(np.float32) < x).astype(np.uint8)).view(f8)
    flf = fl.astype(np.float32)
    gap = ce.astype(np.float32) - flf
    p = np.where(gap > 0, (x - flf) / np.where(gap > 0, gap, 1.0), 0.0)
    rng = np.random.default_rng(seed)
    up = rng.random(x.shape, dtype=np.float32) < p
    return np.where(up, ce, fl).astype(f8)


def _bank_chains(zn, y_true, momentum):
    """Replicate the reference's sequential per-sample EMA scatter (fp32)."""
    valid = (y_true >= 0) & (y_true < N_CLASSES)
    lc = np.clip(y_true, 0, N_CLASSES - 1)
    m = np.float32(momentum)
    one_m = np.float32(1.0 - momentum)
    bank = {}
    for i in np.nonzero(valid)[0]:
        c = int(lc[i])
        if c not in bank:
            bank[c] = zn[i].copy()
        else:
            ema = m * bank[c] + one_m * zn[i]
            n = np.float32(np.sqrt(np.float32((ema ** 2).sum())))
            bank[c] = ema / max(n, np.float32(EPS))
    return bank


def _build_nc(CS):
    """CS = number of 512-wide S-plane chunks (CP = 512*CS classes)."""
    from concourse import bacc, tile, mybir

    dt = mybir.dt
    CP = 512 * CS
    nc = bacc.Bacc("TRN2", target_bir_lowering=False, debug=False)

    zl_d = nc.dram_tensor("zl", (KP, 2 * ROWS), dt.float8e4, kind="ExternalInput")
    rs_d = nc.dram_tensor("rs", (KP, 2 * CP), dt.float8e4, kind="ExternalInput")
    lg_d = nc.dram_tensor("lg", (KP, 2 * NGU * 128), dt.float8e4, kind="ExternalInput")
    rg_d = nc.dram_tensor("rg", (KP, 2 * NGU * 512), dt.float8e4, kind="ExternalInput")
    r1_d = nc.dram_tensor("r1", (128, 4 * CP), dt.float8e4, kind="ExternalInput")
    t2_d = nc.dram_tensor("t2", (128, NGU * 512), dt.float8e4, kind="ExternalInput")
    out_d = nc.dram_tensor("acc_out", (128, 16), dt.float32, kind="ExternalOutput")

    # unit list: (lhs tensor key, lhs col, rhs col, mask col)
    units = []
    for ib in range(4):
        for cc in range(CS):
            units.append(("s", ib * 128, cc * 512, (ib * CS + cc) * 512))
    for u in range(NGU):
        units.append(("g", u * 128, u * 512, u * 512))
    n_units = len(units)
    GSZ = 2                                # units per psum group (2 banks)
    n_groups = (n_units + GSZ - 1) // GSZ

    DR = mybir.MatmulPerfMode.DoubleRow

    with tile.TileContext(nc) as tc:
        with (
            tc.tile_pool(name="const", bufs=1) as constp,
            tc.tile_pool(name="d8p", bufs=3) as d8p,
            tc.tile_pool(name="ep", bufs=3) as ep,
            tc.tile_pool(name="accp", bufs=1) as accp,
            tc.tile_pool(name="psp", bufs=4, space="PSUM") as psp,
        ):
            # resident input tiles ([KP, 2, N] = two k-tiles along free axis)
            zl = constp.tile([KP, 2, ROWS], dt.float8e4, tag="zl")
            rs = constp.tile([KP, 2, CP], dt.float8e4, tag="rs")
            lg = constp.tile([KP, 2, NGU * 128], dt.float8e4, tag="lg")
            rg = constp.tile([KP, 2, NGU * 512], dt.float8e4, tag="rg")
            r1 = constp.tile([128, 4 * CP], dt.float8e4, tag="r1")
            t2 = constp.tile([128, NGU * 512], dt.float8e4, tag="t2")

            bias_t = constp.tile([128, 1], dt.float32)
            acc_all = accp.tile([128, 16], dt.float32)
            nc.gpsimd.memset(bias_t[:], 128.0 + float(DELTA2))
            nc.gpsimd.memset(acc_all[:], 0.0)

            # --- DMA issue ---
            # Transfers share the 16 DMA engines, so concurrent bulk streams
            # starve the critical path; keep one consumption-ordered chain on
            # Sync and only the group-0 critical operands on Scalar.
            # group-0/1 critical operands first, in small chunks
            c0w = min(2048, CP)
            nc.sync.dma_start(rs[:, 0, 0:c0w], rs_d[:, 0:c0w])
            nc.sync.dma_start(rs[:, 1, 0:c0w], rs_d[:, CP:CP + c0w])
            nc.sync.dma_start(zl[:], zl_d[:].rearrange("p (t n) -> p t n", t=2))
            nc.sync.dma_start(r1[:, 0:2048], r1_d[:, 0:2048])
            if c0w < CP:
                nc.sync.dma_start(rs[:, 0, c0w:CP], rs_d[:, c0w:CP])
                nc.sync.dma_start(rs[:, 1, c0w:CP], rs_d[:, CP + c0w:2 * CP])
            nc.sync.dma_start(r1[:, 2048:4096], r1_d[:, 2048:4096])
            # G-plane matmul operands next: the PE reaches them long before
            # the later mask spans are consumed (keeps the HAM clock warm)
            GW = NGU * 512
            nc.sync.dma_start(lg[:], lg_d[:].rearrange("p (t n) -> p t n", t=2))
            nc.sync.dma_start(rg[:, 0, 0:4096], rg_d[:, 0:4096])
            nc.sync.dma_start(rg[:, 1, 0:4096], rg_d[:, GW:GW + 4096])
            nc.sync.dma_start(r1[:, 4096:8192], r1_d[:, 4096:8192])
            nc.sync.dma_start(rg[:, 0, 4096:GW], rg_d[:, 4096:GW])
            nc.sync.dma_start(rg[:, 1, 4096:GW], rg_d[:, GW + 4096:2 * GW])
            nc.sync.dma_start(r1[:, 8192:4 * CP], r1_d[:, 8192:4 * CP])
            nc.sync.dma_start(t2[:, 0:4096], t2_d[:, 0:4096])
            nc.sync.dma_start(t2[:, 4096:8192], t2_d[:, 4096:8192])
            nc.sync.dma_start(t2[:, 8192:GW], t2_d[:, 8192:GW])


            ops = {"s": (zl, rs, r1), "g": (lg, rg, t2)}
            d8t = None
            d8_off = 0
            span_key = None
            span_m0 = 0
            si = 0
            n_s_spans = 0
            for gi in range(n_groups):
                gunits = units[gi * GSZ:(gi + 1) * GSZ]
                gw = 512 * len(gunits)
                key = gunits[0][0]
                ps = psp.tile([128, 512 * GSZ], dt.float32, tag="ps")
                for q, (_, lc0, rc0, mc0) in enumerate(gunits):
                    lt, rt, _ = ops[key]
                    nc.tensor.matmul(
                        ps[:, q * 512:(q + 1) * 512],
                        lt[:, :, lc0:lc0 + 128],
                        rt[:, :, rc0:rc0 + 512],
                        start=True, stop=True, perf_mode=DR,
                    )
                if d8t is None:
                    d8t = d8p.tile([128, 4096], dt.bfloat16, tag="d8")
                    d8_off = 0
                    span_key = key
                    span_m0 = gunits[0][3]
                nc.scalar.activation(
                    d8t[:, d8_off:d8_off + gw], ps[:, 0:gw],
                    mybir.ActivationFunctionType.Sqrt,
                    bias=bias_t[:], scale=64.0,
                )
                d8_off += gw
                nxt = units[(gi + 1) * GSZ:(gi + 1) * GSZ + 1]
                # close the first S span early so VectorE starts sooner
                close = (d8_off >= 4096) or (not nxt) or (nxt[0][0] != span_key) \
                    or (si < 2 and d8_off >= 2048)
                if close:
                    mask = ops[span_key][2]
                    et = ep.tile([128, 4096], dt.bfloat16, tag="et")
                    nc.vector.scalar_tensor_tensor(
                        out=et[:, 0:d8_off],
                        in0=d8t[:, 0:d8_off],
                        scalar=1.0,
                        in1=mask[:, span_m0:span_m0 + d8_off],
                        op0=mybir.AluOpType.mult,
                        op1=mybir.AluOpType.mult,
                        accum_out=acc_all[:, si:si + 1],
                    )
                    if span_key == "s":
                        n_s_spans += 1
                    si += 1
                    d8t = None
            n_spans = si
            assert n_spans <= 16

            nc.sync.dma_start(out_d[:], acc_all[:])

    nc.compile()
    return nc, n_spans, n_s_spans


def _get_nc(CS):
    key = ("nc", CS)
    if key not in _CACHE:
        _CACHE[key] = _build_nc(CS)
    return _CACHE[key]


def kernel(y_true, y_pred, lookup, global_step, current_epoch, _want_trace=False):
    from concourse.bass_utils import run_bass_kernel_spmd

    y_true = np.asarray(y_true).astype(np.int64)
    y_pred = np.asarray(y_pred, dtype=np.float32)
    lookup = np.asarray(lookup, dtype=np.float32)
    gs = int(np.asarray(global_step))

    if gs < MOM_WARMUP:
        momentum = 0.5 + (BASE_MOM - 0.5) * (gs / MOM_WARMUP)
    else:
        momentum = BASE_MOM
    progress = min(1.0, (gs - WARMUP_STEPS) / 5000.0)
    aw = MEMORY_WEIGHT * progress

    # ---- host: normalize, bank scatter-EMA, compaction ----
    z = y_pred[:, :D]
    nrm = np.sqrt((z.astype(np.float64) ** 2).sum(axis=1))
    zn = (z / np.maximum(nrm, EPS)[:, None]).astype(np.float32)

    valid = (y_true >= 0) & (y_true < N_CLASSES)
    bg = ~valid
    nv = int(valid.sum())
    lc = np.clip(y_true, 0, N_CLASSES - 1)

    bank = _bank_chains(zn, y_true, momentum)
    init_list = np.array(sorted(bank.keys()), dtype=np.int64)
    C = len(init_list)
    CS = max(1, (C + 511) // 512)
    CP = 512 * CS

    zn8 = trunc_f8(zn)
    bank_rows = (
        np.stack([bank[c] for c in init_list])
        if C else np.zeros((0, D), np.float32)
    )
    bank8 = trunc_f8(bank_rows)

    znd = zn8.astype(np.float64)
    bankd = bank8.astype(np.float64)

    # ---- host: exact linear terms (fp64) ----
    R = lookup[lc]                    # (B, 8192)
    R_init = R[:, init_list]          # (B, C)
    A_S = 2.0 * nv * C - 2.0 * float(znd[valid].sum(0) @ bankd.sum(0))
    B_S = nv * C - float(R_init[valid].sum(dtype=np.float64))

    T_up = R[:, lc]                   # (B, B): lookup[lc_i, lc_j]
    both_bg = bg[:, None] & bg[None, :]
    one_bg = bg[:, None] ^ bg[None, :]
    T_up = np.where(both_bg, np.float32(BG_SIM),
                    np.where(one_bg, np.float32(BG_OTHER_SIM), T_up))
    # upper-triangle (i<j) oriented pair targets; zero elsewhere
    T_up = np.triu(T_up, 1)

    Np = B * (B - 1) // 2
    szn = znd.sum(0)
    sumG_offdiag = float(szn @ szn) - float((znd ** 2).sum())
    A_G = 2.0 * Np - sumG_offdiag
    B_G = Np - float(T_up.sum(dtype=np.float64))

    # ---- device operand construction (fp8, two 96-row k-tiles) ----
    znT8 = np.ascontiguousarray(zn8.T)                       # (192, B) f8
    znT8m2 = (zn8.astype(np.float32).T * np.float32(-2.0)).astype(f8)
    bankT8m2 = np.zeros((D, CP), dtype=f8)
    if C:
        bankT8m2[:, 0:C] = (
            bank8.astype(np.float32).T * np.float32(-2.0)
        ).astype(f8)

    def ktiles(a):
        """(192, N) -> (96, 2*N): two k-tiles along the free axis."""
        return np.ascontiguousarray(
            np.concatenate([a[0:KP], a[KP:2 * KP]], axis=1)
        )

    in_maps = []
    for core in range(NCORES):
        rbs = CORE_RBS[core]
        rows = np.concatenate([np.arange(rb * 128, rb * 128 + 128) for rb in rbs])

        zl = ktiles(znT8[:, rows])                           # (96, 1024)
        rs = ktiles(bankT8m2)                                # (96, 2*CP)
        r1 = np.zeros((128, 4 * CP), dtype=f8)
        for ib, rb in enumerate(rbs):
            rr = slice(rb * 128, rb * 128 + 128)
            m = (1.0 - R_init[rr]) * valid[rr, None]         # (128, C)
            r1[:, ib * CP:ib * CP + C] = stoch_fp8(m, seed=1000 + rb)

        gunits = [(ib, rb, cc) for ib, rb in enumerate(rbs)
                  for cc in _g_chunks(rb)]
        assert len(gunits) == NGU, (core, len(gunits))

        lgf = np.empty((D, NGU * 128), dtype=f8)
        rgf = np.empty((D, NGU * 512), dtype=f8)
        t2 = np.zeros((128, NGU * 512), dtype=f8)
        for u, (ib, rb, cc) in enumerate(gunits):
            lgf[:, u * 128:(u + 1) * 128] = znT8[:, rb * 128:rb * 128 + 128]
            rgf[:, u * 512:(u + 1) * 512] = znT8m2[:, cc * 512:(cc + 1) * 512]
            blk = 1.0 - T_up[rb * 128:rb * 128 + 128, cc * 512:(cc + 1) * 512]
            jj = np.arange(cc * 512, cc * 512 + 512)[None, :]
            ii = np.arange(rb * 128, rb * 128 + 128)[:, None]
            blk = np.where(jj > ii, blk, 0.0)
            t2[:, u * 512:(u + 1) * 512] = stoch_fp8(blk, seed=2000 + rb * 8 + cc)

        in_maps.append({
            "zl": zl,
            "rs": rs,
            "lg": ktiles(lgf),
            "rg": ktiles(rgf),
            "r1": r1,
            "t2": t2,
        })

    nc, n_spans, n_s_spans = _get_nc(CS)
    if _want_trace:
        import tempfile
        try:
            from trn_agent_boot.trn_boot import _ntff_profile_via_ctypes
            hook = _ntff_profile_via_ctypes("/opt/axon/libaxon_pjrt.so")
            outdir = tempfile.mkdtemp(prefix="ntff_")
            with hook(outdir, [0]):
                res = run_bass_kernel_spmd(nc, in_maps, list(range(NCORES)))
            _CACHE["last_profile_dir"] = outdir
        except Exception as e:
            _CACHE["trace_error"] = repr(e)
            res = run_bass_kernel_spmd(nc, in_maps, list(range(NCORES)))
        _CACHE["last_results"] = res
    else:
        res = run_bass_kernel_spmd(nc, in_maps, list(range(NCORES)))

    P3S = 0.0
    P3G = 0.0
    for r in res.results:
        acc = np.asarray(r["acc_out"], dtype=np.float64)
        P3S += float(acc[:, 0:n_s_spans].sum())
        P3G += float(acc[:, n_s_spans:n_spans].sum())

    mem_sum = A_S + 16.0 * B_S - P3S
    denom = max(nv * C, 1)
    mem_loss = mem_sum / denom
    batch_sum = A_G + 16.0 * B_G - P3G
    batch_loss = batch_sum / Np

    loss = (1.0 - aw) * batch_loss + aw * mem_loss
    return np.float32(loss)
